# revision 23
# baseline (speedup 1.0000x reference)
"""MoE routing kernel (top-2 of 32 experts, dense-mix form) for 8 TRN2 cores.

Math identity used: out = sum_e mix_w[:, e] * (x @ W_e) + mix_b @ expert_biases,
where mix_w / mix_b are the dense top-2 softmax mixture coefficients from the
two routers. Experts are sharded 4-per-core; each core computes a partial sum
(its 4 experts plus its 4 experts' bias rows) and the host adds the 8 partials.

Expert weights stream as fp8 e3m4 (x128 scale, undone in the f32 mix
coefficients), halving the dominant HBM traffic vs bf16: ~4.6 MB/core per
iteration. That moves the bottleneck from the wire to the PE, whose floor is
the weight ingest itself (4 MB / 128 B-per-cycle = 32.7k cycles ~ 13.7 us at
2.4 GHz, plus router/bias/mix rows). The PE multiplies the bf16 x (stationary)
against the fp8 weight stream (moving) directly — mixed-dtype matmul runs at
the same 1 row/cycle.

Structure:
- Weights are staged host-side as [EPC, 128, 2, KT, HD] fp8 so every weight
  DMA has 8 KB-contiguous partition lines (half-expert transfers see 4 KB);
  middle experts ride whole-expert 1 MB transfers to halve HWDGE
  descriptor-gen; the stream tail keeps fine chunks.
- The SP HWDGE ring carries ONLY the input stream (x + weights); outputs and
  small inputs ride the ACT ring, so consecutive reps chain with no gap.
- Router logits come from two PE chains into one psum tile: xtb@rwF (bf16 x,
  fp16 rw — fp16 needs no residual) and xq@(rwF/512) (xq = fp8 e3m4 of
  512*(x - xtb), the /512 copy rides bf16 inside xp), accumulated into one
  psum region that the top-2 mask phase reads directly, making logits
  correct to ~5e-4 (top-2 margins ~1e-3). Near-exactness is
  required: a plain bf16 router flips top-2 selections and costs ~7% output
  error. Expert columns are PERMUTED per core (local experts first), so the
  local mix coefficients are mask-slice DVE ops — no selection matmuls, no
  dense-mix transpose; only the tiny [B,4] bias-mix transpose rides the PE.
- The router chains are emitted first (their inputs land in the previous
  rep's tail), giving the serial DVE softmax chain maximal cover; the
  mix-path PE ops are pinned behind weight arrivals with the PSUM
  round-trip split across blocks: transpose after e1h0 (pinned on w1),
  mixbT's PSUM->SBUF copy on DVE during e1h1 (blended with a w2 slice),
  bias matmuls after e1h1 — the PE never waits on the mix path.
- The last expert's second half streams in shrinking chunks so the final
  matmul + mix-accumulate + output DMA tail after the last byte is minimal.
"""

import sys

if "/opt/trn_rl_repo" not in sys.path:
    sys.path.insert(0, "/opt/trn_rl_repo")

from contextlib import ExitStack

import ml_dtypes
import numpy as np

import concourse.bacc as bacc
import concourse.tile as tile
from concourse import mybir
from concourse.bass_utils import run_bass_kernel_spmd
from concourse.masks import make_identity

B = 128        # batch
D = 1024       # in = out features
E = 32         # experts
NCORES = 8
EPC = E // NCORES   # experts per core
KT = D // 128       # k-tiles of 128 along contraction dim
HD = 512            # psum-bank-sized output chunk
WSCL = 128.0        # fp8 weight scale; undone via the f32 mix coefficients
XSCL = 512.0        # fp8 x-residual scale; undone in the logits combine

F32 = mybir.dt.float32
BF16 = mybir.dt.bfloat16
FP16 = mybir.dt.float16
FP8 = mybir.dt.float8e3
ALU = mybir.AluOpType
ACTF = mybir.ActivationFunctionType


def _ctile(pool, name, shape, dtype):
    # unique tag => dedicated slot, never rotated/reused
    return pool.tile(shape, dtype, name=name, tag=name)


def build_program(reps=1):
    # no partition_id: per-core behavior lives entirely in the input data
    # (per-core expert permutation of xf / weight shards), and dropping the
    # tensor removes the per-engine TENSOR_LOADs from the NEFF preamble
    nc = bacc.Bacc("TRN2", enable_partition_id=False)

    # x pieces: xp bf16 = [x^T | rw/512] (expert stationary + router main
    # term; the rw/512 bf16 copy is chain 2's moving operand — bf16 because
    # rw/512 underflows fp16 normals, and bf16 precision is ample for the
    # correction term — so both chains accumulate into one psum region);
    # xf fp16 = rw (per-core expert-permuted [router_w | bias_router_w]);
    # xq fp8 = 512 * (x - bf16(x)) residual for the router correction chain
    xp_d = nc.dram_tensor("xp", [128, KT, B + 2 * E], BF16, kind="ExternalInput")
    xf_d = nc.dram_tensor("xf", [128, KT, 2 * E], FP16, kind="ExternalInput")
    xq_d = nc.dram_tensor("xq", [128, KT, B], FP8, kind="ExternalInput")
    # fp8 weights, h/k inside the partition line:
    # whs[e, p, h, k, :] = fp8e3(WSCL * W_e[k*128+p, h*HD:(h+1)*HD])
    whs_d = nc.dram_tensor("whs", [EPC, 128, 2, KT, HD], FP8, kind="ExternalInput")
    bscl_d = nc.dram_tensor("bscl", [EPC, D], BF16, kind="ExternalInput")
    out_d = nc.dram_tensor("out", [B, D], BF16, kind="ExternalOutput")

    with ExitStack() as ctx:
        tc = ctx.enter_context(tile.TileContext(nc))
        const = ctx.enter_context(tc.tile_pool(name="const", bufs=1))
        # 3-deep x buffering: with 2, the weave's xp' issue chains on a
        # slot-reuse completion and lands late, stalling the PE ~1 us at
        # some rep boundaries
        xin = ctx.enter_context(tc.tile_pool(name="xin", bufs=3))
        # 2 reps of weight buffering: rep n+1's weight DMAs must never
        # WAR-wait on rep n's expert matmuls, or the wire stalls whenever
        # the PE lags (and the resulting PE idle re-throttles the HAM)
        wpool = ctx.enter_context(tc.tile_pool(name="wts", bufs=8))
        ps_small = ctx.enter_context(tc.tile_pool(name="ps", bufs=1, space="PSUM"))
        ps_e = ctx.enter_context(tc.tile_pool(name="pe", bufs=7, space="PSUM"))

        ident = _ctile(const, "ident", [128, 128], F32)
        make_identity(nc, ident[:])

        def wtile():
            return wpool.tile([128, 2, KT, HD], FP8, name="w", tag="w")

        def make_head():
            # tiles for a rep's stream head: x pieces + expert 0
            xp = xin.tile([128, KT, B + 2 * E], BF16, name="xp", tag="xp")
            xf = xin.tile([128, KT, 2 * E], FP16, name="xf", tag="xf")
            xq = xin.tile([128, KT, B], FP8, name="xq", tag="xq")
            return xp, xf, xq, wtile()

        head = None
        for r in range(reps):
            # ---- SP ring: the input stream. Expert 0's first half rides
            # interleaved with the x chunks so the PE starts on k 0:4 as
            # early as possible; the router inputs follow, then the rest of
            # the weight stream. For reps > 1, the NEXT rep's head is
            # interleaved into this rep's tail so the PE's boundary idle
            # stays under the ~3.4 us HAM re-throttle window. ----
            if head is None:
                head = make_head()
                xp, xf, xq, w0 = head
                nc.sync.dma_start(xp[:, 0:4, :], xp_d[:, 0:4, :])
                nc.sync.dma_start(w0[:, 0, 0:4, :], whs_d[0, :, 0, 0:4, :])
                nc.sync.dma_start(xp[:, 4:8, :], xp_d[:, 4:8, :])
                nc.sync.dma_start(xf[:], xf_d[:])
                nc.sync.dma_start(xq[:], xq_d[:])
                nc.sync.dma_start(w0[:, 0, 4:8, :], whs_d[0, :, 0, 4:8, :])
                nc.sync.dma_start(w0[:, 1, :, :], whs_d[0, :, 1, :, :])
            else:
                xp, xf, xq, w0 = head

            wts = [w0] + [wtile() for _ in range(1, EPC)]
            # middle experts ride as whole-expert transfers (8 KB-contiguous
            # partition lines, half the HWDGE descriptor-gen of per-half
            # chunks); the last expert's first half stays separate so the
            # tail granularity is unchanged. Rep 1 keeps half-expert chunks:
            # its PE rides the wire frontier, and finer arrival granularity
            # cuts the first-rep stalls.
            for e in range(1, EPC - 1):
                if r == 0:
                    nc.sync.dma_start(wts[e][:, 0, :, :], whs_d[e, :, 0, :, :])
                    nc.sync.dma_start(wts[e][:, 1, :, :], whs_d[e, :, 1, :, :])
                else:
                    nc.sync.dma_start(wts[e][:, :, :, :], whs_d[e, :, :, :, :])
            nc.sync.dma_start(
                wts[EPC - 1][:, 0, :, :], whs_d[EPC - 1, :, 0, :, :]
            )

            # next rep's head tiles + first DMAs, woven into this rep's tail
            nhead = make_head() if r + 1 < reps else None
            if nhead is not None:
                nc.sync.dma_start(nhead[0][:], xp_d[:])
                nc.sync.dma_start(nhead[1][:], xf_d[:])
                nc.sync.dma_start(nhead[2][:], xq_d[:])
                nc.sync.dma_start(nhead[3][:, 0, 0:4, :], whs_d[0, :, 0, 0:4, :])
            wl = wts[EPC - 1]
            if nhead is None:
                # final rep: tail chunks shrink so the very last matmuls
                # start (and the output leaves) as soon as possible
                nc.sync.dma_start(wl[:, 1, 0:4, :], whs_d[EPC - 1, :, 1, 0:4, :])
                nc.sync.dma_start(wl[:, 1, 4:6, :], whs_d[EPC - 1, :, 1, 4:6, :])
                nc.sync.dma_start(wl[:, 1, 6:7, :], whs_d[EPC - 1, :, 1, 6:7, :])
                nc.sync.dma_start(wl[:, 1, 7:8, :], whs_d[EPC - 1, :, 1, 7:8, :])
            else:
                # middle reps: finish THIS rep's stream first — the PE is the
                # bottleneck now, and interleaving next-rep chunks before
                # wl[4:8] stalls this rep's last matmuls ~540 ns per boundary
                nc.sync.dma_start(wl[:, 1, 0:4, :], whs_d[EPC - 1, :, 1, 0:4, :])
                nc.sync.dma_start(wl[:, 1, 4:8, :], whs_d[EPC - 1, :, 1, 4:8, :])
                nc.sync.dma_start(nhead[3][:, 0, 4:8, :], whs_d[0, :, 0, 4:8, :])
            if nhead is not None:
                nc.sync.dma_start(nhead[3][:, 1, :, :], whs_d[0, :, 1, :, :])
            head = nhead

            # ---- ACT ring: small inputs (outputs join it at the end) ----
            bscl = _ctile(const, "bscl", [EPC, D], BF16)
            nc.scalar.dma_start(bscl[:], bscl_d[:])

            # ---- router + expert-0 first half. Steady reps run the router
            # FIRST (xf/xq land in the previous rep's tail, so the DVE
            # softmax chain starts ~1.7 us earlier and beats the PE to the
            # pinned transpose). Rep 1 runs e0h0 first: its xf/xq only land
            # ~2.5 us in, and an in-order PE stuck behind the router would
            # idle past e0h0's inputs (~1.3 us), wasting full-rate HBM
            # window the run's tail needs. Router logits, near-exact
            # (~5e-4, margins ~1e-3): two chains accumulate into ONE psum
            # region (chain 2's moving operand is the pre-scaled rw/512
            # copy, undoing xq's 512):
            # logits = sum_k xtb@rwF + sum_k xq@(rwF/512)
            pl = ps_small.tile([B, 2 * E], F32, name="ps")
            pe0 = [ps_e.tile([B, HD], F32, name="pe") for _ in range(2)]

            def emit_router():
                for k in range(KT):
                    nc.tensor.matmul(
                        pl[:], xp[:, k, 0:B], xf[:, k, 0:2 * E],
                        start=(k == 0), stop=False,
                    )
                for k in range(KT):
                    nc.tensor.matmul(
                        pl[:], xq[:, k, 0:B], xp[:, k, B:B + 2 * E],
                        start=False, stop=(k == KT - 1),
                    )

            def emit_e0h0():
                for k in range(KT):
                    nc.tensor.matmul(
                        pe0[0][:], xp[:, k, 0:B], wts[0][:, 0, k, :],
                        start=(k == 0), stop=(k == KT - 1),
                    )

            if r == 0:
                emit_e0h0()
                emit_router()
            else:
                emit_router()
                emit_e0h0()
            logits = pl

            # ---- top-2 + softmax per half -> local mix coeffs. Expert
            # columns are per-core permuted so the local 4 experts are cols
            # 0:EPC of each half; coefficients come straight off the top-2
            # masks, no dense-mix or selection matmuls. The two halves'
            # max/mask phases run first so one [B, 2] ACTIVATE covers both
            # exps — one ACT round-trip instead of two. ----
            dgap = _ctile(const, "dgap", [B, 2], F32)
            ed = _ctile(const, "ed", [B, 2], F32)
            m1s, m2s = [], []
            for h in range(2):
                lh = logits[:, h * E:(h + 1) * E]
                mx1 = _ctile(const, f"mx1_{h}", [B, 1], F32)
                nc.vector.tensor_reduce(mx1[:], lh, axis=mybir.AxisListType.X, op=ALU.max)
                m1 = _ctile(const, f"m1_{h}", [B, E], F32)
                nc.vector.tensor_scalar(m1[:], lh, mx1[:], None, op0=ALU.is_ge)
                msk = _ctile(const, f"msk_{h}", [B, E], F32)
                nc.vector.scalar_tensor_tensor(
                    msk[:], m1[:], -1e30, lh, op0=ALU.mult, op1=ALU.add
                )
                mx2 = _ctile(const, f"mx2_{h}", [B, 1], F32)
                nc.vector.tensor_reduce(mx2[:], msk[:], axis=mybir.AxisListType.X, op=ALU.max)
                m2 = _ctile(const, f"m2_{h}", [B, E], F32)
                nc.vector.tensor_scalar(m2[:], msk[:], mx2[:], None, op0=ALU.is_ge)
                nc.vector.tensor_sub(dgap[:, h:h + 1], mx2[:], mx1[:])
                m1s.append(m1)
                m2s.append(m2)
            nc.scalar.activation(ed[:], dgap[:], ACTF.Exp)
            # h=0 -> weight-mix coeffs [B, EPC] f32 (1/WSCL folded in);
            # h=1 -> bias-mix coeffs [B, EPC] bf16 (transposed on PE below)
            mix_loc = _ctile(const, "mix_loc", [B, EPC], F32)
            mixb_pre = _ctile(const, "mixb_pre", [B, EPC], F32)
            for h, dst, dscl in ((0, mix_loc, WSCL), (1, mixb_pre, 1.0)):
                # den = (ed + 1) * dscl, so 1/WSCL rides the reciprocal
                # (dscl is a power of two -> exact)
                den = _ctile(const, f"den_{h}", [B, 1], F32)
                nc.vector.tensor_scalar(
                    den[:], ed[:, h:h + 1], dscl, dscl, op0=ALU.mult, op1=ALU.add
                )
                p1 = _ctile(const, f"p1_{h}", [B, 1], F32)
                nc.vector.reciprocal(p1[:], den[:])
                p2 = _ctile(const, f"p2_{h}", [B, 1], F32)
                nc.vector.tensor_mul(p2[:], ed[:, h:h + 1], p1[:])
                t2 = _ctile(const, f"t2_{h}", [B, EPC], F32)
                nc.vector.tensor_scalar_mul(t2[:], m2s[h][:, 0:EPC], p2[:])
                nc.vector.scalar_tensor_tensor(
                    dst[:], m1s[h][:, 0:EPC], p1[:], t2[:],
                    op0=ALU.mult, op1=ALU.add,
                )

            # ---- expert 0 second half ----
            for k in range(KT):
                nc.tensor.matmul(
                    pe0[1][:], xp[:, k, 0:B], wts[0][:, 1, k, :],
                    start=(k == 0), stop=(k == KT - 1),
                )

            # ---- expert 1 first half ----
            pe1 = [ps_e.tile([B, HD], F32, name="pe") for _ in range(2)]
            for k in range(KT):
                nc.tensor.matmul(
                    pe1[0][:], xp[:, k, 0:B], wts[1][:, 0, k, :],
                    start=(k == 0), stop=(k == KT - 1),
                )

            # The mix-path PE ops must sort AFTER the weight-fed expert
            # matmul blocks in the in-order PE queue: the scheduler's
            # cost-model sim otherwise pulls them right behind the router
            # (it underestimates when the expert weights land), and their
            # softmax wait then stalls the PE. Priorities can't fix this
            # (ready work dispatches to a sim-idle engine immediately), so
            # pin with REAL dependencies, split to hide the PSUM round-trip:
            # - transpose pinned on w1 (identity blended with the w1 tile)
            #   -> runs right after e1h0; the router-first softmax chain has
            #   mixb_pre ready by then;
            # - mixbT's PSUM->SBUF copy is a no-op blend with a w2 slice
            #   -> runs on DVE while the PE streams e1h1;
            # - bias matmuls follow e1h1 -> mixbT already in SBUF, no PE
            #   stall between transpose and bias.
            ident2 = _ctile(const, "ident2", [128, 128], F32)
            nc.vector.scalar_tensor_tensor(
                ident2[:], wts[1][:, 1, 0, 0:128], 0.0, ident[:],
                op0=ALU.mult, op1=ALU.add,
            )
            ptm = ps_small.tile([EPC, B], F32, name="ps")
            nc.tensor.transpose(ptm[:], mixb_pre[:], ident2[:])
            mixbT = _ctile(const, "mixbT", [EPC, B], BF16)
            nc.vector.scalar_tensor_tensor(
                mixbT[:], wts[2][0:EPC, 1, 0, 0:B], 0.0, ptm[:],
                op0=ALU.mult, op1=ALU.add,
            )

            # ---- expert 1 second half ----
            for k in range(KT):
                nc.tensor.matmul(
                    pe1[1][:], xp[:, k, 0:B], wts[1][:, 1, k, :],
                    start=(k == 0), stop=(k == KT - 1),
                )


            # ---- local bias term: mixb_loc @ bscl -> [B, D] ----
            pb = [ps_e.tile([B, HD], F32, name="pe") for _ in range(2)]
            bias_sb = _ctile(const, "bias_sb", [B, D], F32)
            for h in range(2):
                nc.tensor.matmul(
                    pb[h][:], mixbT[:], bscl[:, h * HD:(h + 1) * HD],
                    start=True, stop=True,
                )
                nc.vector.tensor_copy(bias_sb[:, h * HD:(h + 1) * HD], pb[h][:])

            # ---- mix-accumulate chain: acc_e = pe_e * mix_e + acc_{e-1},
            # seeded with the bias ----
            acc0 = _ctile(const, "acc0", [B, D], F32)
            acc1 = _ctile(const, "acc1", [B, D], F32)
            for h in range(2):
                hs, he = h * HD, (h + 1) * HD
                nc.vector.scalar_tensor_tensor(
                    acc0[:, hs:he], pe0[h][:], mix_loc[:, 0:1],
                    bias_sb[:, hs:he], op0=ALU.mult, op1=ALU.add,
                )
                nc.vector.scalar_tensor_tensor(
                    acc1[:, hs:he], pe1[h][:], mix_loc[:, 1:2],
                    acc0[:, hs:he], op0=ALU.mult, op1=ALU.add,
                )

            # ---- experts 2 and 3 ----
            prev = acc1
            for e in range(2, EPC):
                last = e == EPC - 1
                pe = [ps_e.tile([B, HD], F32, name="pe") for _ in range(2)]
                # bf16 final accumulator: host sums the 8 partials in f64
                acc = _ctile(const, f"acc{e}", [B, D], BF16 if last else F32)
                for h in range(2):
                    hs, he = h * HD, (h + 1) * HD
                    for k in range(KT):
                        nc.tensor.matmul(
                            pe[h][:], xp[:, k, 0:B], wts[e][:, h, k, :],
                            start=(k == 0), stop=(k == KT - 1),
                        )
                    nc.vector.scalar_tensor_tensor(
                        acc[:, hs:he], pe[h][:], mix_loc[:, e:e + 1],
                        prev[:, hs:he], op0=ALU.mult, op1=ALU.add,
                    )
                    if last:
                        # middle reps: outputs ride SWDGE (gpsimd) — HWDGE
                        # completion lanes are shared round-robin by BOTH
                        # rings, so a compute-gated output DMA on a lane
                        # stalls the next rep's input DMA on that lane. The
                        # 8 DMA-SW lanes are a separate pool. The final rep
                        # has nothing left to stall, so its outputs take the
                        # lower-latency HWDGE path out.
                        oeng = nc.scalar if r == reps - 1 else nc.gpsimd
                        oeng.dma_start(out_d[:, hs:he], acc[:, hs:he])
                prev = acc

    nc.finalize()
    return nc


def make_input_maps(x, router_w, bias_router_w, expert_weights, expert_biases):
    xt = np.ascontiguousarray(
        x.T.reshape(KT, 128, B).transpose(1, 0, 2), dtype=np.float32
    )
    xtb = xt.astype(ml_dtypes.bfloat16)
    xq = np.ascontiguousarray(
        ((xt - xtb.astype(np.float32)) * XSCL).astype(ml_dtypes.float8_e3m4)
    )
    rw = (
        np.concatenate([router_w, bias_router_w], axis=1)
        .reshape(KT, 128, 2 * E)
        .transpose(1, 0, 2)
        .astype(np.float32)
    )

    in_maps = []
    for c in range(NCORES):
        # per-core expert permutation: this core's experts first, so the
        # local mix coefficients are fixed mask columns 0:EPC on device
        perm = list(range(c * EPC, (c + 1) * EPC)) + [
            e for e in range(E) if not (c * EPC <= e < (c + 1) * EPC)
        ]
        rwp = np.concatenate(
            [rw[:, :, perm], rw[:, :, [E + p for p in perm]]], axis=2
        )
        xf = np.ascontiguousarray(rwp.astype(np.float16))
        xp = np.ascontiguousarray(np.concatenate(
            [xtb, (rwp / XSCL).astype(ml_dtypes.bfloat16)], axis=2
        ))
        we8 = (
            (np.asarray(expert_weights[c * EPC:(c + 1) * EPC], np.float32) * WSCL)
            .astype(ml_dtypes.float8_e3m4)
            .reshape(EPC, KT, 128, 2, HD)
            .transpose(0, 2, 3, 1, 4)
        )
        whs = np.ascontiguousarray(we8)
        bscl = np.ascontiguousarray(
            expert_biases[c * EPC:(c + 1) * EPC]
        ).astype(ml_dtypes.bfloat16)
        in_maps.append(dict(xp=xp, xf=xf, xq=xq, whs=whs, bscl=bscl))
    return in_maps


def kernel(x, router_w, bias_router_w, expert_weights, expert_biases, **bench_kwargs):
    in_maps = make_input_maps(x, router_w, bias_router_w, expert_weights, expert_biases)
    nc = build_program()
    res = run_bass_kernel_spmd(nc, in_maps, list(range(NCORES)), **bench_kwargs)
    out = np.zeros((B, D), dtype=np.float64)
    for r in res.results:
        out += r["out"].astype(np.float64)
    final = out.astype(np.float32)
    if bench_kwargs:
        kernel.last_result = res
    return final
